# revision 9
# baseline (speedup 1.0000x reference)
"""Multi-head attention (with the repo's k=q bug) on 8 Trainium2 NeuronCores.

Reference computation (B=2, S=2048, D=512, H=8, DK=64):
    q = query @ Wq.T ; v = value @ Wv.T          (k-projection is dead code)
    qh = q.reshape(B, H, S, DK)  (raw view: head h = a contiguous 256-row slab
                                  of q, re-chunked into rows of 64)
    kh = qh                      (repo bug: key = query.view(...))
    scores = qh @ qh^T / 8 ; mask ; softmax ; x = attn @ vh
    out = x.transpose/reshape @ Wo.T
Sharding: core c owns head h=c for both batches (2 (b,h) pairs/core).

v3 layout (ACT-bound design): the scalar engine runs ONLY the 40 exp
activations (312ns + 1.0ns/col each); every copy/evac lives on vector/
gpsimd/DMA.  Inputs stream in k-tile-sliced so the q projection starts
after the first 128-contraction slice lands; q is projected in two
row-halves straight into the j-ordered qhT layout via strided PSUM->SBUF
casts (no intermediate qc stage).  Diagonal j-tiles store b1's surviving
columns immediately after b0's (b1-shift), so one contiguous exp covers
exactly the unmasked work.  Per (j-tile t, i-chunk n), score-transposed:
    E_T = exp(S_T/8 - 20) * tri-mask
    [x_unnorm^T; l] += [vh | ones]^T augmented PV matmul   (per batch)
Chunk epilogue: po = x_unnorm^T.T @ Wo_h.T written bf16; host divides by l
and sums partials over heads/cores.  Fully-masked j-tiles are skipped and
diagonal tiles are column-restricted (causal structure verified on host;
non-causal masks fall back to numpy).
"""

import math
import sys

import numpy as np

sys.path.insert(0, "/opt/trn_rl_repo")

B, S, D, H, DK = 2, 2048, 512, 8, 64
NCORES = 8
SLAB = S // H          # 256 query rows per head-slab
CHUNK = 512            # i-chunk width
JT = 128               # j-tile height
NCHUNK = S // CHUNK    # 4
NJT = S // JT          # 16
KT = D // 128          # 4 k-tiles over the projections' contraction dim
EXP_BIAS = -20.0


def _patch_walrus_logging():
    from concourse import bass_utils
    if getattr(bass_utils, "_ldw_patched", False):
        return
    orig = bass_utils.run_command

    def run_command(argv, **kw):
        import subprocess
        try:
            return orig(argv, **kw)
        except subprocess.CalledProcessError as e:
            err = e.stderr if isinstance(e.stderr, str) else (
                e.stderr.decode() if e.stderr else "")
            out = e.stdout if isinstance(e.stdout, str) else (
                e.stdout.decode() if e.stdout else "")
            sys.stderr.write("WALRUS FAIL STDERR:\n" + err[-4000:] + "\n")
            sys.stderr.write("WALRUS FAIL STDOUT:\n" + out[-4000:] + "\n")
            raise

    bass_utils.run_command = run_command
    bass_utils._ldw_patched = True

_cache: dict = {}


def _build_causal():
    import concourse.bass as bass
    import concourse.tile as tile
    from concourse import bacc, mybir

    _patch_walrus_logging()

    f32 = mybir.dt.float32
    bf16 = mybir.dt.bfloat16
    nc = bacc.Bacc("TRN2", target_bir_lowering=False, debug=False,
                   num_devices=NCORES)

    # inputs (all bf16; batches packed side by side in the free dim)
    qT = nc.dram_tensor("qT", [D, 2 * SLAB], bf16, kind="ExternalInput").ap()
    vT = nc.dram_tensor("vT", [D, 2 * SLAB], bf16, kind="ExternalInput").ap()
    wqT = nc.dram_tensor("wqT", [D, D], bf16, kind="ExternalInput").ap()
    wvT = nc.dram_tensor("wvT", [D, D], bf16, kind="ExternalInput").ap()
    woT = nc.dram_tensor("woT", [DK, D], bf16, kind="ExternalInput").ap()
    mtri = nc.dram_tensor("mtri", [JT, JT], bf16, kind="ExternalInput").ap()
    po = nc.dram_tensor("po", [B, S, D], bf16, kind="ExternalOutput").ap()
    lo = nc.dram_tensor("lo", [B, 1, S], f32, kind="ExternalOutput").ap()

    with tile.TileContext(nc) as tc:
        with (
            tc.tile_pool(name="const", bufs=1) as constp,
            tc.tile_pool(name="acts", bufs=1) as actp,
            tc.tile_pool(name="qhT", bufs=1) as qhTp,
            tc.tile_pool(name="vh", bufs=1) as vhp,
            tc.tile_pool(name="eT", bufs=4) as eTp,
            tc.tile_pool(name="xT", bufs=2) as xTp,
            tc.tile_pool(name="fo", bufs=2) as fop,
            tc.tile_pool(name="psS", bufs=2, space="PSUM") as psS,
            tc.tile_pool(name="psX", bufs=1, space="PSUM") as psX,
            tc.tile_pool(name="psM", bufs=1, space="PSUM") as psM,
        ):
            # ---- constants / memsets first (cheap engine ops) -----------
            exp_bias = constp.tile([128, 1], f32, tag="ebias")
            nc.gpsimd.memset(exp_bias[:], EXP_BIAS)
            vh_all = []
            for b in range(B):
                t = vhp.tile([128, NJT * (DK + 1)], bf16, tag=f"vha{b}",
                             name=f"vha{b}")
                vv = t.rearrange("p (t c) -> p t c", c=DK + 1)
                nc.gpsimd.memset(vv[:, :, DK:DK + 1], 1.0)
                vh_all.append(t)
            vh_v = [t.rearrange("p (t c) -> p t c", c=DK + 1) for t in vh_all]

            # ---- k-sliced critical input DMAs ---------------------------
            # wq/qT arrive one 128-row contraction slice at a time so the
            # projection k-loop starts after the first slice lands.  Queue
            # plan keeps transfer order = consumption order per queue.
            wq_k = [constp.tile([128, D], bf16, tag=f"wq{k}", name=f"wq{k}")
                    for k in range(KT)]
            qT_k = [actp.tile([128, 2 * SLAB], bf16, tag=f"qt{k}",
                              name=f"qt{k}")
                    for k in range(KT)]
            for k in range(KT):
                qt_eng = nc.scalar if k % 2 == 0 else nc.gpsimd
                nc.sync.dma_start(wq_k[k][:], wqT[128 * k:128 * (k + 1), :])
                qt_eng.dma_start(qT_k[k][:], qT[128 * k:128 * (k + 1), :])

            wv_k = [constp.tile([128, D], bf16, tag=f"wv{k}", name=f"wv{k}")
                    for k in range(KT)]
            vT_k = [actp.tile([128, 2 * SLAB], bf16, tag=f"vt{k}",
                              name=f"vt{k}")
                    for k in range(KT)]
            wo_sb = constp.tile([128, D], bf16, tag="wo")
            mt_sb = constp.tile([JT, JT], bf16, tag="mtri")

            # ---- PE warm-up: dummy matmuls into the (still unused) psx
            # banks while the critical input loads are in flight ----------
            dmy = constp.tile([128, D], bf16, tag="dmy")
            nc.gpsimd.memset(dmy[:], 0.0)
            for i in range(14):
                psd = psX.tile([DK + 1, CHUNK], f32, tag=f"psx{i % 2}")
                nc.tensor.matmul(psd[:], dmy[:, 0:DK + 1], dmy[:],
                                 start=True, stop=True)

            # ---- secondary input loads: ride the same queues so they
            # start only after the critical qT/wq slices on that queue ----
            for k in range(KT):
                nc.gpsimd.dma_start(wv_k[k][:], wvT[128 * k:128 * (k + 1), :])
                nc.scalar.dma_start(vT_k[k][:], vT[128 * k:128 * (k + 1), :])
            nc.sync.dma_start(wo_sb[0:64, :], woT[:, :])
            nc.sync.dma_start(wo_sb[64:128, :], woT[:, :])
            nc.sync.dma_start(mt_sb[:], mtri[:, :])

            # ---- q projection in slab-row halves, straight into the
            # j-ordered qhT layout (col j = 8*r + c) ----------------------
            # half hh covers slab rows r in [128*hh, 128*(hh+1)) for both
            # batches -> qhT_h[hh] cols 0:1024 (j = 8r+c local).
            qhT_h = [qhTp.tile([128, 8 * 128], bf16, tag=f"qhT{hh}",
                              name=f"qhT{hh}")
                     for hh in range(2)]

            def qproj_half(hh):
                psq = psS.tile([128, 2 * CHUNK], f32, tag="pss")
                pv = psq.rearrange("p (jg b r) -> p jg b r", jg=4, b=2)
                for k in range(KT):
                    for jg in range(4):
                        nc.tensor.matmul(
                            pv[:, jg, :, :],
                            wq_k[k][:, 128 * jg:128 * (jg + 1)],
                            qT_k[k].rearrange("p (b r) -> p b r", b=2)
                            [:, :, 128 * hh:128 * (hh + 1)],
                            start=(k == 0), stop=(k == KT - 1))
                # evac: 4 strided casts (b x col-parity), partition-shifted
                # where the psq half and the batch half differ
                dstv = qhT_h[hh].rearrange(
                    "p (r jg cp) -> p cp jg r", jg=4, cp=2)
                for b in range(2):
                    for par in range(2):
                        src = pv[64 * par:64 * (par + 1), :, b, :]
                        dst = dstv[64 * b:64 * (b + 1), par, :, :]
                        nc.vector.tensor_copy(dst, src)

            qproj_half(0)

            def qhT_stat(t_):
                # stationary slice for j-tile t_ (both batch halves)
                return qhT_h[t_ // 8][:, 128 * (t_ % 8):128 * (t_ % 8 + 1)]

            def qhT_mov(n, lo_, hi_):
                # moving slice: chunk n cols lo_:hi_ (within-chunk)
                return qhT_h[n // 2][:, 512 * (n % 2) + lo_:
                                     512 * (n % 2) + hi_]

            # ---- v projection halves + vh gathers -----------------------
            vsl = {}

            def vproj(rhs):
                for rh in rhs:
                    psv = psM.tile([128, D], f32, tag=f"psf{rh % 2}")
                    b, half = rh // 2, rh % 2
                    for k in range(KT):
                        nc.tensor.matmul(
                            psv[:],
                            vT_k[k][:, 256 * b + 128 * half:
                                    256 * b + 128 * (half + 1)],
                            wv_k[k][:], start=(k == 0), stop=(k == KT - 1))
                    vc = actp.tile([128, D], bf16, tag=f"vsl{rh}")
                    nc.vector.tensor_copy(vc[:], psv[:])
                    vsl[rh] = vc

            def vgather(half):
                # dst partition jj = 8*rm + c8 <- vsl[rh][16*tl+rm, 64*c8+d]
                for tl in range(8):
                    for b in range(2):
                        rh = 2 * b + half
                        t_ = 8 * half + tl
                        src = vsl[rh].rearrange(
                            "(tl rm) f -> tl rm f", tl=8)[tl]
                        nc.sync.dma_start(vh_v[b][:, t_, 0:DK], src)

            # lo accumulators (flushed at the very end)
            lacc0 = xTp.tile([1, S], f32, tag="lacc0", bufs=1)
            lacc1 = xTp.tile([1, S], f32, tag="lacc1", bufs=1)

            # ---- attention: epilogues interleaved into next chunk -------
            def make_epilogue(n, psx0, psx1, final=False):
                thunks = []

                def t_evac():
                    xT = xTp.tile([128, CHUNK], bf16, tag="xT")
                    nc.vector.tensor_copy(xT[0:64, :], psx0[0:64, :])
                    nc.vector.tensor_copy(xT[64:128, :], psx1[0:64, :])
                    nc.vector.tensor_copy(
                        lacc1[:, CHUNK * n:CHUNK * (n + 1)],
                        psx1[64:65, :])
                    nc.vector.tensor_copy(
                        lacc0[:, CHUNK * n:CHUNK * (n + 1)], psx0[64:65, :])
                    if final:
                        nc.sync.dma_start(lo[0, :, :], lacc0[:])
                        nc.gpsimd.dma_start(lo[1, :, :], lacc1[:])
                    fo0 = fop.tile([128, 4 * D], bf16, tag="fo0")
                    fo1 = fop.tile([128, 4 * D], bf16, tag="fo1")
                    make_epilogue.state = (xT, fo0, fo1)
                thunks.append(t_evac)

                def t_proj(u):
                    def run():
                        xT, fo0, fo1 = make_epilogue.state
                        if final and u % 2 == 1:
                            ps = psS.tile([128, 2 * CHUNK], f32, tag="pss")
                            psf0 = ps[:, 0:D]
                            psf1 = ps[:, D:2 * D]
                        else:
                            psf0 = psM.tile([128, D], f32, tag="psf0")
                            psf1 = psM.tile([128, D], f32, tag="psf1")
                        nc.tensor.matmul(
                            psf0[:], xT[0:64, 128 * u:128 * (u + 1)],
                            wo_sb[0:64, :], start=True, stop=True,
                            tile_position=(0, 0))
                        nc.tensor.matmul(
                            psf1[:], xT[64:128, 128 * u:128 * (u + 1)],
                            wo_sb[64:128, :], start=True, stop=True,
                            tile_position=(64, 0))
                        nc.vector.tensor_copy(
                            fo0[:, D * u:D * (u + 1)], psf0[:])
                        nc.vector.tensor_copy(
                            fo1[:, D * u:D * (u + 1)], psf1[:])
                        if final:
                            r0 = CHUNK * n + 128 * u
                            e0 = nc.sync if u % 2 == 0 else nc.gpsimd
                            e1 = nc.gpsimd if u % 2 == 0 else nc.sync
                            e0.dma_start(
                                po[0, r0:r0 + 128, :], fo0[:, D * u:D * (u + 1)])
                            e1.dma_start(
                                po[1, r0:r0 + 128, :], fo1[:, D * u:D * (u + 1)])
                    return run
                for u in range(4):
                    thunks.append(t_proj(u))

                def t_store():
                    if final:
                        return
                    xT, fo0, fo1 = make_epilogue.state
                    for b, fo in ((0, fo0), (1, fo1)):
                        dst = po[b, CHUNK * n:CHUNK * (n + 1), :].rearrange(
                            "(u p) c -> p u c", u=4)
                        eng = nc.sync if b == 0 else nc.gpsimd
                        eng.dma_start(
                            dst, fo.rearrange("p (u c) -> p u c", u=4))
                thunks.append(t_store)
                return thunks

            pending = []
            for n in (0, 1, 2, 3):
                n_t = 4 * n + 4
                psx0 = psX.tile([DK + 1, CHUNK], f32, tag="psx0")
                psx1 = psX.tile([DK + 1, CHUNK], f32, tag="psx1")
                for t_ in range(n_t):
                    s_ = t_ - 4 * n
                    off = max(0, s_) * JT
                    # b1-shift: b1's surviving cols start right at CHUNK,
                    # so exp covers one gap-free range [off : 2*CHUNK-off]
                    pss = psS.tile([128, 2 * CHUNK], f32, tag="pss")
                    nc.tensor.matmul(
                        pss[:, off:CHUNK],
                        qhT_stat(t_)[0:64, :],
                        qhT_mov(n, off, CHUNK)[0:64, :],
                        start=True, stop=True, tile_position=(0, 0))
                    nc.tensor.matmul(
                        pss[:, CHUNK:2 * CHUNK - off],
                        qhT_stat(t_)[64:128, :],
                        qhT_mov(n, off, CHUNK)[64:128, :],
                        start=True, stop=True, tile_position=(64, 0))
                    eT = eTp.tile([128, 2 * CHUNK], bf16, tag="eT")
                    nc.scalar.activation(
                        eT[:, off:2 * CHUNK - off], pss[:, off:2 * CHUNK - off],
                        mybir.ActivationFunctionType.Exp,
                        bias=exp_bias[:], scale=1.0 / math.sqrt(DK))
                    if s_ >= 0:
                        sl0 = eT[:, off:off + JT]
                        nc.gpsimd.tensor_mul(sl0, sl0, mt_sb[:])
                        sl1 = eT[:, CHUNK:CHUNK + JT]
                        nc.gpsimd.tensor_mul(sl1, sl1, mt_sb[:])
                    if n == 0 and t_ == 0:
                        # first-half v projection: after chunk 0's first
                        # score matmul (so exp starts ASAP) but ahead of
                        # the first PV in the PE's program order
                        vproj((0, 2))
                        vgather(0)
                    elif n == 1 and t_ == 0:
                        vproj((3,))
                        vgather(1)
                    elif n == 1 and t_ == 1:
                        qproj_half(1)
                    nc.tensor.matmul(
                        psx0[:, off:], vh_all[0][:, 65 * t_:65 * t_ + 65],
                        eT[:, off:CHUNK],
                        start=(t_ == 0), stop=(t_ == n_t - 1),
                        skip_group_check=True)
                    nc.tensor.matmul(
                        psx1[:, off:], vh_all[1][:, 65 * t_:65 * t_ + 65],
                        eT[:, CHUNK:2 * CHUNK - off],
                        start=(t_ == 0), stop=(t_ == n_t - 1),
                        skip_group_check=True)
                    if pending:
                        pending.pop(0)()
                if n == 0:
                    # second-half v projection split: rh1 here, rh3 inside
                    # chunk 1's first tile (smaller PE bursts)
                    vproj((1,))
                for th in pending:
                    th()
                pending = make_epilogue(n, psx0, psx1, final=(n == 3))
            for th in pending:
                th()
    nc.compile()
    return nc


def _mask_patterns():
    import ml_dtypes
    p = np.arange(JT)[:, None]
    f = np.arange(JT)[None, :]
    return (p <= f).astype(ml_dtypes.bfloat16)


def _numpy_fallback(query, key, value, mask, Wq, Wk, Wv, Wo):
    q = query @ Wq.T
    v = value @ Wv.T
    qh = q.reshape(B, H, S, DK)
    vh = v.reshape(B, H, S, DK)
    scores = np.einsum("bhqd,bhkd->bhqk", qh, qh) / math.sqrt(DK)
    scores = np.where(mask == 0, np.float32(-1e9), scores)
    scores = scores - scores.max(axis=-1, keepdims=True)
    e = np.exp(scores)
    attn = e / e.sum(axis=-1, keepdims=True)
    x = np.einsum("bhqk,bhkd->bhqd", attn, vh)
    x = x.transpose(0, 2, 1, 3).reshape(B, S, H * DK)
    return (x @ Wo.T).astype(np.float32)


def _run_device(query, value, Wq, Wv, Wo, trace=False):
    import ml_dtypes
    from concourse.bass_utils import run_bass_kernel_spmd

    if "nc" not in _cache:
        _cache["nc"] = _build_causal()
    nc = _cache["nc"]

    bf = ml_dtypes.bfloat16
    mtri = _mask_patterns()
    wqT = np.ascontiguousarray(Wq.T).astype(bf)
    wvT = np.ascontiguousarray(Wv.T).astype(bf)
    in_maps = []
    for c in range(NCORES):
        r0 = SLAB * c
        qs = query[:, r0:r0 + SLAB, :]      # [B, SLAB, D]
        vs = value[:, r0:r0 + SLAB, :]
        in_maps.append({
            # [D, 2*SLAB]: b0 cols then b1 cols
            "qT": np.ascontiguousarray(
                qs.transpose(2, 0, 1).reshape(D, 2 * SLAB)).astype(bf),
            "vT": np.ascontiguousarray(
                vs.transpose(2, 0, 1).reshape(D, 2 * SLAB)).astype(bf),
            "wqT": wqT,
            "wvT": wvT,
            "woT": np.ascontiguousarray(
                Wo[:, DK * c:DK * (c + 1)].T).astype(bf),
            "mtri": mtri,
        })
    res = run_bass_kernel_spmd(nc, in_maps, core_ids=list(range(NCORES)),
                               trace=trace)
    out = np.zeros((B, S, D), dtype=np.float32)
    for c in range(NCORES):
        pc = res.results[c]
        out += pc["po"].astype(np.float32) / \
            pc["lo"].reshape(B, S, 1)
    return out, res


_TRIL = None


def kernel(query, key, value, mask, Wq, Wk, Wv, Wo):
    global _TRIL
    query = np.asarray(query, dtype=np.float32)
    value = np.asarray(value, dtype=np.float32)
    mask = np.asarray(mask)
    Wq = np.asarray(Wq, dtype=np.float32)
    Wv = np.asarray(Wv, dtype=np.float32)
    Wo = np.asarray(Wo, dtype=np.float32)

    if _TRIL is None:
        _TRIL = np.tril(np.ones((S, S), dtype=np.int64))
    m2 = mask.reshape(S, S)
    if not np.array_equal(m2 != 0, _TRIL != 0):
        return _numpy_fallback(query, np.asarray(key), value, mask,
                               Wq, np.asarray(Wk), Wv, Wo)

    out, _ = _run_device(query, value, Wq, Wv, Wo)
    return out


# revision 11
# speedup vs baseline: 1.2287x; 1.2287x over previous
"""Multi-head attention (with the repo's k=q bug) on 8 Trainium2 NeuronCores.

Reference computation (B=2, S=2048, D=512, H=8, DK=64):
    q = query @ Wq.T ; v = value @ Wv.T          (k-projection is dead code)
    qh = q.reshape(B, H, S, DK)  (raw view: head h = a contiguous 256-row slab
                                  of q, re-chunked into rows of 64)
    kh = qh                      (repo bug: key = query.view(...))
    scores = qh @ qh^T / 8 ; mask ; softmax ; x = attn @ vh
    out = x.transpose/reshape @ Wo.T
Sharding: core c owns head h=c for both batches (2 (b,h) pairs/core).

v3 layout (ACT-bound design): the scalar engine runs ONLY the 40 exp
activations (312ns + 1.0ns/col each); every copy/evac lives on vector/
gpsimd/DMA.  Inputs stream in k-tile-sliced so the q projection starts
after the first 128-contraction slice lands; q is projected in two
row-halves straight into the j-ordered qhT layout via strided PSUM->SBUF
casts (no intermediate qc stage).  Diagonal j-tiles store b1's surviving
columns immediately after b0's (b1-shift), so one contiguous exp covers
exactly the unmasked work.  Per (j-tile t, i-chunk n), score-transposed:
    E_T = exp(S_T/8 - 20) * tri-mask
    [x_unnorm^T; l] += [vh | ones]^T augmented PV matmul   (per batch)
Chunk epilogue: po = x_unnorm^T.T @ Wo_h.T written bf16; host divides by l
and sums partials over heads/cores.  Fully-masked j-tiles are skipped and
diagonal tiles are column-restricted (causal structure verified on host;
non-causal masks fall back to numpy).
"""

import math
import sys

import numpy as np

sys.path.insert(0, "/opt/trn_rl_repo")

B, S, D, H, DK = 2, 2048, 512, 8, 64
NCORES = 8
SLAB = S // H          # 256 query rows per head-slab
CHUNK = 512            # i-chunk width
JT = 128               # j-tile height
NCHUNK = S // CHUNK    # 4
NJT = S // JT          # 16
KT = D // 128          # 4 k-tiles over the projections' contraction dim
EXP_BIAS = -20.0


def _patch_walrus_logging():
    from concourse import bass_utils
    if getattr(bass_utils, "_ldw_patched", False):
        return
    orig = bass_utils.run_command

    def run_command(argv, **kw):
        import subprocess
        try:
            return orig(argv, **kw)
        except subprocess.CalledProcessError as e:
            err = e.stderr if isinstance(e.stderr, str) else (
                e.stderr.decode() if e.stderr else "")
            out = e.stdout if isinstance(e.stdout, str) else (
                e.stdout.decode() if e.stdout else "")
            sys.stderr.write("WALRUS FAIL STDERR:\n" + err[-4000:] + "\n")
            sys.stderr.write("WALRUS FAIL STDOUT:\n" + out[-4000:] + "\n")
            raise

    bass_utils.run_command = run_command
    bass_utils._ldw_patched = True

_cache: dict = {}


def _build_causal():
    import concourse.bass as bass
    import concourse.tile as tile
    from concourse import bacc, mybir

    _patch_walrus_logging()

    f32 = mybir.dt.float32
    bf16 = mybir.dt.bfloat16
    nc = bacc.Bacc("TRN2", target_bir_lowering=False, debug=False,
                   num_devices=NCORES)

    # inputs (all bf16; batches packed side by side in the free dim)
    qT = nc.dram_tensor("qT", [D, 2 * SLAB], bf16, kind="ExternalInput").ap()
    vT = nc.dram_tensor("vT", [D, 2 * SLAB], bf16, kind="ExternalInput").ap()
    wqT = nc.dram_tensor("wqT", [D, D], bf16, kind="ExternalInput").ap()
    wvT = nc.dram_tensor("wvT", [D, D], bf16, kind="ExternalInput").ap()
    woT = nc.dram_tensor("woT", [DK, D], bf16, kind="ExternalInput").ap()
    mtri = nc.dram_tensor("mtri", [JT, JT], bf16, kind="ExternalInput").ap()
    po = nc.dram_tensor("po", [B, S, D], bf16, kind="ExternalOutput").ap()
    lo = nc.dram_tensor("lo", [B, 1, S], f32, kind="ExternalOutput").ap()

    with tile.TileContext(nc) as tc:
        with (
            tc.tile_pool(name="const", bufs=1) as constp,
            tc.tile_pool(name="acts", bufs=1) as actp,
            tc.tile_pool(name="qhT", bufs=1) as qhTp,
            tc.tile_pool(name="vh", bufs=1) as vhp,
            tc.tile_pool(name="eT", bufs=4) as eTp,
            tc.tile_pool(name="xT", bufs=2) as xTp,
            tc.tile_pool(name="fo", bufs=2) as fop,
            tc.tile_pool(name="psS", bufs=2, space="PSUM") as psS,
            tc.tile_pool(name="psX", bufs=1, space="PSUM") as psX,
            tc.tile_pool(name="psM", bufs=1, space="PSUM") as psM,
        ):
            # ---- constants / memsets first (cheap engine ops) -----------
            exp_bias = constp.tile([128, 1], f32, tag="ebias")
            nc.gpsimd.memset(exp_bias[:], EXP_BIAS)
            vh_all = []
            for b in range(B):
                t = vhp.tile([128, NJT * (DK + 1)], bf16, tag=f"vha{b}",
                             name=f"vha{b}")
                vv = t.rearrange("p (t c) -> p t c", c=DK + 1)
                nc.gpsimd.memset(vv[:, :, DK:DK + 1], 1.0)
                vh_all.append(t)
            vh_v = [t.rearrange("p (t c) -> p t c", c=DK + 1) for t in vh_all]

            # ---- k-sliced critical input DMAs ---------------------------
            # wq/qT arrive one 128-row contraction slice at a time so the
            # projection k-loop starts after the first slice lands.  Queue
            # plan keeps transfer order = consumption order per queue.
            wq_k = [constp.tile([128, D], bf16, tag=f"wq{k}", name=f"wq{k}")
                    for k in range(KT)]
            qT_k = [actp.tile([128, 2 * SLAB], bf16, tag=f"qt{k}",
                              name=f"qt{k}")
                    for k in range(KT)]
            for k in range(KT):
                qt_eng = nc.scalar if k % 2 == 0 else nc.gpsimd
                nc.sync.dma_start(wq_k[k][:], wqT[128 * k:128 * (k + 1), :])
                qt_eng.dma_start(qT_k[k][:], qT[128 * k:128 * (k + 1), :])

            wv_k = [constp.tile([128, D], bf16, tag=f"wv{k}", name=f"wv{k}")
                    for k in range(KT)]
            vT_k = [actp.tile([128, 2 * SLAB], bf16, tag=f"vt{k}",
                              name=f"vt{k}")
                    for k in range(KT)]
            wo_sb = constp.tile([128, D], bf16, tag="wo")
            mt_sb = constp.tile([JT, JT], bf16, tag="mtri")

            # ---- PE warm-up: dummy matmuls into the (still unused) psx
            # banks while the critical input loads are in flight ----------
            dmy = constp.tile([128, D], bf16, tag="dmy")
            nc.gpsimd.memset(dmy[:], 0.0)
            for i in range(14):
                psd = psX.tile([DK + 1, CHUNK], f32, tag=f"psx{i % 2}")
                nc.tensor.matmul(psd[:], dmy[:, 0:DK + 1], dmy[:],
                                 start=True, stop=True)

            # ---- secondary input loads: ride the same queues so they
            # start only after the critical qT/wq slices on that queue ----
            for k in range(KT):
                nc.gpsimd.dma_start(wv_k[k][:], wvT[128 * k:128 * (k + 1), :])
                nc.scalar.dma_start(vT_k[k][:], vT[128 * k:128 * (k + 1), :])
            nc.sync.dma_start(wo_sb[0:64, :], woT[:, :])
            nc.sync.dma_start(wo_sb[64:128, :], woT[:, :])
            nc.sync.dma_start(mt_sb[:], mtri[:, :])

            # ---- q projection in slab-row halves, straight into the
            # j-ordered qhT layout (col j = 8*r + c) ----------------------
            # half hh covers slab rows r in [128*hh, 128*(hh+1)) for both
            # batches -> qhT_h[hh] cols 0:1024 (j = 8r+c local).
            qhT_h = [qhTp.tile([128, 8 * 128], bf16, tag=f"qhT{hh}",
                              name=f"qhT{hh}")
                     for hh in range(2)]

            def qproj_half(hh):
                # psq col = 512*b + 128*jg + r  (1-D moving slices per b)
                psq = psS.tile([128, 2 * CHUNK], f32, tag="pss")
                pv = psq.rearrange("p (b jg r) -> p b jg r", b=2, jg=4)
                for b in range(2):
                    for jg in range(4):
                        for k in range(KT):
                            nc.tensor.matmul(
                                pv[:, b, jg, :],
                                wq_k[k][:, 128 * jg:128 * (jg + 1)],
                                qT_k[k][:, 256 * b + 128 * hh:
                                        256 * b + 128 * (hh + 1)],
                                start=(k == 0), stop=(k == KT - 1))
                # evac: one contiguous cast to SBUF, then 4 strided scatter
                # copies (b x col-parity) into the j-ordered layout
                qtmp = actp.tile([128, 2 * CHUNK], bf16, tag="qtmp",
                                 name="qtmp")
                nc.vector.tensor_copy(qtmp[:], psq[:])
                tv = qtmp.rearrange("p (b jg r) -> p b jg r", b=2, jg=4)
                dstv = qhT_h[hh].rearrange(
                    "p (r jg cp) -> p cp jg r", jg=4, cp=2)
                for b in range(2):
                    for par in range(2):
                        src = tv[64 * par:64 * (par + 1), b, :, :]
                        dst = dstv[64 * b:64 * (b + 1), par, :, :]
                        eng = nc.gpsimd if (b + par) % 2 == 0 else nc.vector
                        eng.tensor_copy(dst, src)

            qproj_half(0)

            def qhT_stat(t_):
                # stationary slice for j-tile t_ (both batch halves)
                return qhT_h[t_ // 8][:, 128 * (t_ % 8):128 * (t_ % 8 + 1)]

            def qhT_mov(n, lo_, hi_):
                # moving slice: chunk n cols lo_:hi_ (within-chunk)
                return qhT_h[n // 2][:, 512 * (n % 2) + lo_:
                                     512 * (n % 2) + hi_]

            # ---- v projection halves + vh gathers -----------------------
            vsl = {}

            def vproj(rhs):
                for rh in rhs:
                    psv = psM.tile([128, D], f32, tag=f"psf{rh % 2}")
                    b, half = rh // 2, rh % 2
                    for k in range(KT):
                        nc.tensor.matmul(
                            psv[:],
                            vT_k[k][:, 256 * b + 128 * half:
                                    256 * b + 128 * (half + 1)],
                            wv_k[k][:], start=(k == 0), stop=(k == KT - 1))
                    vc = actp.tile([128, D], bf16, tag=f"vsl{rh}")
                    nc.vector.tensor_copy(vc[:], psv[:])
                    vsl[rh] = vc

            def vgather(half):
                # dst partition jj = 8*rm + c8 <- vsl[rh][16*tl+rm, 64*c8+d]
                for tl in range(8):
                    for b in range(2):
                        rh = 2 * b + half
                        t_ = 8 * half + tl
                        src = vsl[rh].rearrange(
                            "(tl rm) f -> tl rm f", tl=8)[tl]
                        nc.sync.dma_start(vh_v[b][:, t_, 0:DK], src)

            # lo accumulators (flushed at the very end)
            lacc0 = xTp.tile([1, S], f32, tag="lacc0", bufs=1)
            lacc1 = xTp.tile([1, S], f32, tag="lacc1", bufs=1)

            # ---- attention: epilogues interleaved into next chunk -------
            def make_epilogue(n, psx0, psx1, final=False):
                thunks = []

                def t_evac():
                    xT = xTp.tile([128, CHUNK], bf16, tag="xT")
                    nc.vector.tensor_copy(xT[0:64, :], psx0[0:64, :])
                    nc.vector.tensor_copy(xT[64:128, :], psx1[0:64, :])
                    nc.vector.tensor_copy(
                        lacc1[:, CHUNK * n:CHUNK * (n + 1)],
                        psx1[64:65, :])
                    nc.vector.tensor_copy(
                        lacc0[:, CHUNK * n:CHUNK * (n + 1)], psx0[64:65, :])
                    if final:
                        nc.sync.dma_start(lo[0, :, :], lacc0[:])
                        nc.gpsimd.dma_start(lo[1, :, :], lacc1[:])
                    fo0 = fop.tile([128, 4 * D], bf16, tag="fo0")
                    fo1 = fop.tile([128, 4 * D], bf16, tag="fo1")
                    make_epilogue.state = (xT, fo0, fo1)
                thunks.append(t_evac)

                def t_proj(u):
                    def run():
                        xT, fo0, fo1 = make_epilogue.state
                        if final and u % 2 == 1:
                            ps = psS.tile([128, 2 * CHUNK], f32, tag="pss")
                            psf0 = ps[:, 0:D]
                            psf1 = ps[:, D:2 * D]
                        else:
                            psf0 = psM.tile([128, D], f32, tag="psf0")
                            psf1 = psM.tile([128, D], f32, tag="psf1")
                        nc.tensor.matmul(
                            psf0[:], xT[0:64, 128 * u:128 * (u + 1)],
                            wo_sb[0:64, :], start=True, stop=True,
                            tile_position=(0, 0))
                        nc.tensor.matmul(
                            psf1[:], xT[64:128, 128 * u:128 * (u + 1)],
                            wo_sb[64:128, :], start=True, stop=True,
                            tile_position=(64, 0))
                        nc.vector.tensor_copy(
                            fo0[:, D * u:D * (u + 1)], psf0[:])
                        nc.vector.tensor_copy(
                            fo1[:, D * u:D * (u + 1)], psf1[:])
                        if final:
                            r0 = CHUNK * n + 128 * u
                            e0 = nc.sync if u % 2 == 0 else nc.gpsimd
                            e1 = nc.gpsimd if u % 2 == 0 else nc.sync
                            e0.dma_start(
                                po[0, r0:r0 + 128, :], fo0[:, D * u:D * (u + 1)])
                            e1.dma_start(
                                po[1, r0:r0 + 128, :], fo1[:, D * u:D * (u + 1)])
                    return run
                for u in range(4):
                    thunks.append(t_proj(u))

                def t_store():
                    if final:
                        return
                    xT, fo0, fo1 = make_epilogue.state
                    for b, fo in ((0, fo0), (1, fo1)):
                        dst = po[b, CHUNK * n:CHUNK * (n + 1), :].rearrange(
                            "(u p) c -> p u c", u=4)
                        eng = nc.sync if b == 0 else nc.gpsimd
                        eng.dma_start(
                            dst, fo.rearrange("p (u c) -> p u c", u=4))
                thunks.append(t_store)
                return thunks

            pending = []
            for n in (0, 1, 2, 3):
                n_t = 4 * n + 4
                psx0 = psX.tile([DK + 1, CHUNK], f32, tag="psx0")
                psx1 = psX.tile([DK + 1, CHUNK], f32, tag="psx1")
                for t_ in range(n_t):
                    s_ = t_ - 4 * n
                    off = max(0, s_) * JT
                    # b1-shift: b1's surviving cols start right at CHUNK,
                    # so exp covers one gap-free range [off : 2*CHUNK-off]
                    pss = psS.tile([128, 2 * CHUNK], f32, tag="pss")
                    nc.tensor.matmul(
                        pss[:, off:CHUNK],
                        qhT_stat(t_)[0:64, :],
                        qhT_mov(n, off, CHUNK)[0:64, :],
                        start=True, stop=True, tile_position=(0, 0))
                    nc.tensor.matmul(
                        pss[:, CHUNK:2 * CHUNK - off],
                        qhT_stat(t_)[64:128, :],
                        qhT_mov(n, off, CHUNK)[64:128, :],
                        start=True, stop=True, tile_position=(64, 0))
                    eT = eTp.tile([128, 2 * CHUNK], bf16, tag="eT")
                    nc.scalar.activation(
                        eT[:, off:2 * CHUNK - off], pss[:, off:2 * CHUNK - off],
                        mybir.ActivationFunctionType.Exp,
                        bias=exp_bias[:], scale=1.0 / math.sqrt(DK))
                    if s_ >= 0:
                        sl0 = eT[:, off:off + JT]
                        nc.gpsimd.tensor_mul(sl0, sl0, mt_sb[:])
                        sl1 = eT[:, CHUNK:CHUNK + JT]
                        nc.gpsimd.tensor_mul(sl1, sl1, mt_sb[:])
                    if n == 0 and t_ == 0:
                        # first-half v projection: after chunk 0's first
                        # score matmul (so exp starts ASAP) but ahead of
                        # the first PV in the PE's program order
                        vproj((0, 2))
                        vgather(0)
                    elif n == 1 and t_ == 0:
                        vproj((3,))
                        vgather(1)
                    elif n == 1 and t_ == 1:
                        qproj_half(1)
                    nc.tensor.matmul(
                        psx0[:, off:], vh_all[0][:, 65 * t_:65 * t_ + 65],
                        eT[:, off:CHUNK],
                        start=(t_ == 0), stop=(t_ == n_t - 1),
                        skip_group_check=True)
                    nc.tensor.matmul(
                        psx1[:, off:], vh_all[1][:, 65 * t_:65 * t_ + 65],
                        eT[:, CHUNK:2 * CHUNK - off],
                        start=(t_ == 0), stop=(t_ == n_t - 1),
                        skip_group_check=True)
                    if pending:
                        pending.pop(0)()
                if n == 0:
                    # second-half v projection split: rh1 here, rh3 inside
                    # chunk 1's first tile (smaller PE bursts)
                    vproj((1,))
                for th in pending:
                    th()
                pending = make_epilogue(n, psx0, psx1, final=(n == 3))
            for th in pending:
                th()
    nc.compile()
    return nc


def _mask_patterns():
    import ml_dtypes
    p = np.arange(JT)[:, None]
    f = np.arange(JT)[None, :]
    return (p <= f).astype(ml_dtypes.bfloat16)


def _numpy_fallback(query, key, value, mask, Wq, Wk, Wv, Wo):
    q = query @ Wq.T
    v = value @ Wv.T
    qh = q.reshape(B, H, S, DK)
    vh = v.reshape(B, H, S, DK)
    scores = np.einsum("bhqd,bhkd->bhqk", qh, qh) / math.sqrt(DK)
    scores = np.where(mask == 0, np.float32(-1e9), scores)
    scores = scores - scores.max(axis=-1, keepdims=True)
    e = np.exp(scores)
    attn = e / e.sum(axis=-1, keepdims=True)
    x = np.einsum("bhqk,bhkd->bhqd", attn, vh)
    x = x.transpose(0, 2, 1, 3).reshape(B, S, H * DK)
    return (x @ Wo.T).astype(np.float32)


def _run_device(query, value, Wq, Wv, Wo, trace=False):
    import ml_dtypes
    from concourse.bass_utils import run_bass_kernel_spmd

    if "nc" not in _cache:
        _cache["nc"] = _build_causal()
    nc = _cache["nc"]

    bf = ml_dtypes.bfloat16
    mtri = _mask_patterns()
    wqT = np.ascontiguousarray(Wq.T).astype(bf)
    wvT = np.ascontiguousarray(Wv.T).astype(bf)
    in_maps = []
    for c in range(NCORES):
        r0 = SLAB * c
        qs = query[:, r0:r0 + SLAB, :]      # [B, SLAB, D]
        vs = value[:, r0:r0 + SLAB, :]
        in_maps.append({
            # [D, 2*SLAB]: b0 cols then b1 cols
            "qT": np.ascontiguousarray(
                qs.transpose(2, 0, 1).reshape(D, 2 * SLAB)).astype(bf),
            "vT": np.ascontiguousarray(
                vs.transpose(2, 0, 1).reshape(D, 2 * SLAB)).astype(bf),
            "wqT": wqT,
            "wvT": wvT,
            "woT": np.ascontiguousarray(
                Wo[:, DK * c:DK * (c + 1)].T).astype(bf),
            "mtri": mtri,
        })
    res = run_bass_kernel_spmd(nc, in_maps, core_ids=list(range(NCORES)),
                               trace=trace)
    out = np.zeros((B, S, D), dtype=np.float32)
    for c in range(NCORES):
        pc = res.results[c]
        out += pc["po"].astype(np.float32) / \
            pc["lo"].reshape(B, S, 1)
    return out, res


_TRIL = None


def kernel(query, key, value, mask, Wq, Wk, Wv, Wo):
    global _TRIL
    query = np.asarray(query, dtype=np.float32)
    value = np.asarray(value, dtype=np.float32)
    mask = np.asarray(mask)
    Wq = np.asarray(Wq, dtype=np.float32)
    Wv = np.asarray(Wv, dtype=np.float32)
    Wo = np.asarray(Wo, dtype=np.float32)

    if _TRIL is None:
        _TRIL = np.tril(np.ones((S, S), dtype=np.int64))
    m2 = mask.reshape(S, S)
    if not np.array_equal(m2 != 0, _TRIL != 0):
        return _numpy_fallback(query, np.asarray(key), value, mask,
                               Wq, np.asarray(Wk), Wv, Wo)

    out, _ = _run_device(query, value, Wq, Wv, Wo)
    return out


# revision 17
# speedup vs baseline: 1.3180x; 1.0727x over previous
"""Multi-head attention (with the repo's k=q bug) on 8 Trainium2 NeuronCores.

Reference computation (B=2, S=2048, D=512, H=8, DK=64):
    q = query @ Wq.T ; v = value @ Wv.T          (k-projection is dead code)
    qh = q.reshape(B, H, S, DK)  (raw view: head h = a contiguous 256-row slab
                                  of q, re-chunked into rows of 64)
    kh = qh                      (repo bug: key = query.view(...))
    scores = qh @ qh^T / 8 ; mask ; softmax ; x = attn @ vh
    out = x.transpose/reshape @ Wo.T
Sharding: core c owns head h=c for both batches (2 (b,h) pairs/core).

v3 layout (ACT-bound design): the scalar engine runs ONLY the 40 exp
activations (312ns + 1.0ns/col each); every copy/evac lives on vector/
gpsimd/DMA.  Inputs stream in k-tile-sliced so the q projection starts
after the first 128-contraction slice lands; q is projected in two
row-halves straight into the j-ordered qhT layout via strided PSUM->SBUF
casts (no intermediate qc stage).  Diagonal j-tiles store b1's surviving
columns immediately after b0's (b1-shift), so one contiguous exp covers
exactly the unmasked work.  Per (j-tile t, i-chunk n), score-transposed:
    E_T = exp(S_T/8 - 20) * tri-mask
    [x_unnorm^T; l] += [vh | ones]^T augmented PV matmul   (per batch)
Chunk epilogue: po = x_unnorm^T.T @ Wo_h.T written bf16; host divides by l
and sums partials over heads/cores.  Fully-masked j-tiles are skipped and
diagonal tiles are column-restricted (causal structure verified on host;
non-causal masks fall back to numpy).
"""

import math
import sys

import numpy as np

sys.path.insert(0, "/opt/trn_rl_repo")

B, S, D, H, DK = 2, 2048, 512, 8, 64
NCORES = 8
SLAB = S // H          # 256 query rows per head-slab
CHUNK = 512            # i-chunk width
JT = 128               # j-tile height
NCHUNK = S // CHUNK    # 4
NJT = S // JT          # 16
KT = D // 128          # 4 k-tiles over the projections' contraction dim
EXP_BIAS = -20.0


def _patch_walrus_logging():
    from concourse import bass_utils
    if getattr(bass_utils, "_ldw_patched", False):
        return
    orig = bass_utils.run_command

    def run_command(argv, **kw):
        import subprocess
        try:
            return orig(argv, **kw)
        except subprocess.CalledProcessError as e:
            err = e.stderr if isinstance(e.stderr, str) else (
                e.stderr.decode() if e.stderr else "")
            out = e.stdout if isinstance(e.stdout, str) else (
                e.stdout.decode() if e.stdout else "")
            sys.stderr.write("WALRUS FAIL STDERR:\n" + err[-4000:] + "\n")
            sys.stderr.write("WALRUS FAIL STDOUT:\n" + out[-4000:] + "\n")
            raise

    bass_utils.run_command = run_command
    bass_utils._ldw_patched = True

_cache: dict = {}


def _build_causal():
    import concourse.bass as bass
    import concourse.tile as tile
    from concourse import bacc, mybir

    _patch_walrus_logging()

    f32 = mybir.dt.float32
    bf16 = mybir.dt.bfloat16
    nc = bacc.Bacc("TRN2", target_bir_lowering=False, debug=False,
                   num_devices=NCORES)

    # inputs (all bf16; batches packed side by side in the free dim)
    qT = nc.dram_tensor("qT", [D, 2 * SLAB], bf16, kind="ExternalInput").ap()
    vT = nc.dram_tensor("vT", [D, 2 * SLAB], bf16, kind="ExternalInput").ap()
    wqT = nc.dram_tensor("wqT", [D, D], bf16, kind="ExternalInput").ap()
    wvT = nc.dram_tensor("wvT", [D, D], bf16, kind="ExternalInput").ap()
    woT = nc.dram_tensor("woT", [DK, D], bf16, kind="ExternalInput").ap()
    mtri = nc.dram_tensor("mtri", [JT, JT], bf16, kind="ExternalInput").ap()
    po = nc.dram_tensor("po", [B, S, D], bf16, kind="ExternalOutput").ap()
    lo = nc.dram_tensor("lo", [B, 1, S], f32, kind="ExternalOutput").ap()

    with tile.TileContext(nc) as tc:
        with (
            tc.tile_pool(name="const", bufs=1) as constp,
            tc.tile_pool(name="acts", bufs=1) as actp,
            tc.tile_pool(name="qhT", bufs=1) as qhTp,
            tc.tile_pool(name="vh", bufs=1) as vhp,
            tc.tile_pool(name="eT", bufs=4) as eTp,
            tc.tile_pool(name="xT", bufs=2) as xTp,
            tc.tile_pool(name="fo", bufs=2) as fop,
            tc.tile_pool(name="psS", bufs=2, space="PSUM") as psS,
            tc.tile_pool(name="psX", bufs=1, space="PSUM") as psX,
            tc.tile_pool(name="psM", bufs=1, space="PSUM") as psM,
        ):
            # ---- constants / memsets first (cheap engine ops) -----------
            exp_bias = constp.tile([128, 1], f32, tag="ebias")
            nc.gpsimd.memset(exp_bias[:], EXP_BIAS)
            vh_all = []
            for b in range(B):
                t = vhp.tile([128, NJT * (DK + 1)], bf16, tag=f"vha{b}",
                             name=f"vha{b}")
                vv = t.rearrange("p (t c) -> p t c", c=DK + 1)
                nc.gpsimd.memset(vv[:, :, DK:DK + 1], 1.0)
                vh_all.append(t)
            vh_v = [t.rearrange("p (t c) -> p t c", c=DK + 1) for t in vh_all]

            # ---- k-sliced critical input DMAs ---------------------------
            # wq/qT arrive one 128-row contraction slice at a time so the
            # projection k-loop starts after the first slice lands.  Queue
            # plan keeps transfer order = consumption order per queue.
            wq_k = [constp.tile([128, D], bf16, tag=f"wq{k}", name=f"wq{k}")
                    for k in range(KT)]
            qT_k = [actp.tile([128, 2 * SLAB], bf16, tag=f"qt{k}",
                              name=f"qt{k}")
                    for k in range(KT)]
            for k in range(KT):
                qt_eng = nc.scalar if k % 2 == 0 else nc.gpsimd
                nc.sync.dma_start(wq_k[k][:], wqT[128 * k:128 * (k + 1), :])
                qt_eng.dma_start(qT_k[k][:], qT[128 * k:128 * (k + 1), :])

            wv_k = [constp.tile([128, D], bf16, tag=f"wv{k}", name=f"wv{k}")
                    for k in range(KT)]
            vT_k = [actp.tile([128, 2 * SLAB], bf16, tag=f"vt{k}",
                              name=f"vt{k}")
                    for k in range(KT)]
            wo_sb = constp.tile([128, D], bf16, tag="wo")
            mt_sb = constp.tile([JT, JT], bf16, tag="mtri")

            # ---- PE warm-up: dummy matmuls into the (still unused) psx
            # banks while the critical input loads are in flight ----------
            dmy = constp.tile([128, D], bf16, tag="dmy")
            nc.gpsimd.memset(dmy[:], 0.0)
            for i in range(7):
                psd = psX.tile([DK + 1, CHUNK], f32, tag=f"psx{i % 2}")
                nc.tensor.matmul(psd[:], dmy[:, 0:DK + 1], dmy[:],
                                 start=True, stop=True)

            # ---- secondary input loads: ride the same queues so they
            # start only after the critical qT/wq slices on that queue ----
            for k in range(KT):
                nc.gpsimd.dma_start(wv_k[k][:], wvT[128 * k:128 * (k + 1), :])
                nc.scalar.dma_start(vT_k[k][:], vT[128 * k:128 * (k + 1), :])
            nc.sync.dma_start(wo_sb[0:64, :], woT[:, :])
            nc.sync.dma_start(wo_sb[64:128, :], woT[:, :])
            nc.sync.dma_start(mt_sb[:], mtri[:, :])

            # ---- q projection in slab-row halves, straight into the
            # j-ordered qhT layout (col j = 8*r + c) ----------------------
            # half hh covers slab rows r in [128*hh, 128*(hh+1)) for both
            # batches -> qhT_h[hh] cols 0:1024 (j = 8r+c local).
            qhT_h = [qhTp.tile([128, 8 * 128], bf16, tag=f"qhT{hh}",
                              name=f"qhT{hh}")
                     for hh in range(2)]

            def qproj_half(hh):
                # psq col = 512*b + 128*jg + r  (1-D moving slices per b)
                psq = psS.tile([128, 2 * CHUNK], f32, tag="pss")
                pv = psq.rearrange("p (b jg r) -> p b jg r", b=2, jg=4)
                for b in range(2):
                    for jg in range(4):
                        for k in range(KT):
                            nc.tensor.matmul(
                                pv[:, b, jg, :],
                                wq_k[k][:, 128 * jg:128 * (jg + 1)],
                                qT_k[k][:, 256 * b + 128 * hh:
                                        256 * b + 128 * (hh + 1)],
                                start=(k == 0), stop=(k == KT - 1))
                # evac: one contiguous cast to SBUF, then 4 scatter copies
                # into the c-major permuted layout (16-contiguous runs).
                # Stored col (within half) = th*128 + jg*32 + par*16 + rm,
                # true j offset = 8*(16*th+rm) + 2*jg + par; the j/i
                # permutation p' = c8*16+rm <-> 8*rm+c8 is undone by the
                # po/lo store APs and a permuted tri-mask.
                qtmp = actp.tile([128, 2 * CHUNK], bf16, tag="qtmp",
                                 name="qtmp")
                nc.vector.tensor_copy(qtmp[:], psq[:])
                tv = qtmp.rearrange("p (b jg th rm) -> p b jg th rm",
                                    b=2, jg=4, th=8)
                dstv = qhT_h[hh].rearrange(
                    "p (th jg cp rm) -> p cp jg th rm", th=8, jg=4, cp=2)
                for b in range(2):
                    for par in range(2):
                        src = tv[64 * par:64 * (par + 1), b, :, :, :]
                        dst = dstv[64 * b:64 * (b + 1), par, :, :, :]
                        eng = nc.gpsimd if (b + par) % 2 == 0 else nc.vector
                        eng.tensor_copy(dst, src)

            qproj_half(0)

            def qhT_stat(t_):
                # stationary slice for j-tile t_ (both batch halves)
                return qhT_h[t_ // 8][:, 128 * (t_ % 8):128 * (t_ % 8 + 1)]

            def qhT_mov(n, lo_, hi_):
                # moving slice: chunk n cols lo_:hi_ (within-chunk)
                return qhT_h[n // 2][:, 512 * (n % 2) + lo_:
                                     512 * (n % 2) + hi_]

            # ---- v projection halves + vh gathers -----------------------
            vsl = {}

            def vproj(rhs):
                for rh in rhs:
                    psv = psM.tile([128, D], f32, tag=f"psf{rh % 2}")
                    b, half = rh // 2, rh % 2
                    for k in range(KT):
                        nc.tensor.matmul(
                            psv[:],
                            vT_k[k][:, 256 * b + 128 * half:
                                    256 * b + 128 * (half + 1)],
                            wv_k[k][:], start=(k == 0), stop=(k == KT - 1))
                    vc = actp.tile([128, D], bf16, tag=f"vsl{rh}")
                    nc.vector.tensor_copy(vc[:], psv[:])
                    vsl[rh] = vc

            def vgather(half):
                # dst partition jj = 8*rm + c8 <- vsl[rh][16*tl+rm, 64*c8+d]
                for tl in range(8):
                    for b in range(2):
                        rh = 2 * b + half
                        t_ = 8 * half + tl
                        src = vsl[rh].rearrange(
                            "(tl rm) f -> tl rm f", tl=8)[tl]
                        dst = vh_v[b][:, t_, 0:DK].rearrange(
                            "(c8 rm) dk -> rm c8 dk", c8=8)
                        nc.sync.dma_start(dst, src)

            # lo accumulators (flushed at the very end)
            lacc0 = xTp.tile([1, S], f32, tag="lacc0", bufs=1)
            lacc1 = xTp.tile([1, S], f32, tag="lacc1", bufs=1)

            # ---- attention: epilogues interleaved into next chunk -------
            def make_epilogue(n, psx0, psx1, final=False):
                thunks = []

                def t_evac():
                    xT = xTp.tile([128, CHUNK], bf16, tag="xT")
                    nc.vector.tensor_copy(xT[0:64, :], psx0[0:64, :])
                    if final:
                        nc.scalar.copy(xT[64:128, :], psx1[0:64, :])
                        nc.scalar.copy(
                            lacc1[:, CHUNK * n:CHUNK * (n + 1)],
                            psx1[64:65, :])
                    else:
                        nc.vector.tensor_copy(xT[64:128, :], psx1[0:64, :])
                        nc.vector.tensor_copy(
                            lacc1[:, CHUNK * n:CHUNK * (n + 1)],
                            psx1[64:65, :])
                    nc.vector.tensor_copy(
                        lacc0[:, CHUNK * n:CHUNK * (n + 1)], psx0[64:65, :])
                    if final:
                        # lo stays in stored (permuted) order; host undoes it
                        nc.sync.dma_start(lo[0, :, :], lacc0[:])
                        nc.gpsimd.dma_start(lo[1, :, :], lacc1[:])
                    fo0 = fop.tile([128, 4 * D], bf16, tag="fo0")
                    fo1 = fop.tile([128, 4 * D], bf16, tag="fo1")
                    make_epilogue.state = (xT, fo0, fo1)
                thunks.append(t_evac)

                def t_proj(u):
                    def run():
                        xT, fo0, fo1 = make_epilogue.state
                        if final and u % 2 == 1:
                            ps = psS.tile([128, 2 * CHUNK], f32, tag="pss")
                            psf0 = ps[:, 0:D]
                            psf1 = ps[:, D:2 * D]
                        else:
                            psf0 = psM.tile([128, D], f32, tag="psf0")
                            psf1 = psM.tile([128, D], f32, tag="psf1")
                        nc.tensor.matmul(
                            psf0[:], xT[0:64, 128 * u:128 * (u + 1)],
                            wo_sb[0:64, :], start=True, stop=True,
                            tile_position=(0, 0))
                        nc.tensor.matmul(
                            psf1[:], xT[64:128, 128 * u:128 * (u + 1)],
                            wo_sb[64:128, :], start=True, stop=True,
                            tile_position=(64, 0))
                        nc.vector.tensor_copy(
                            fo0[:, D * u:D * (u + 1)], psf0[:])
                        if final:
                            nc.scalar.copy(
                                fo1[:, D * u:D * (u + 1)], psf1[:])
                        else:
                            nc.vector.tensor_copy(
                                fo1[:, D * u:D * (u + 1)], psf1[:])
                        if final:
                            r0 = CHUNK * n + 128 * u
                            e0 = nc.sync if u % 2 == 0 else nc.gpsimd
                            e1 = nc.gpsimd if u % 2 == 0 else nc.sync
                            e0.dma_start(
                                po[0, r0:r0 + 128, :].rearrange(
                                    "(rm c8) c -> c8 rm c", c8=8),
                                fo0[:, D * u:D * (u + 1)])
                            e1.dma_start(
                                po[1, r0:r0 + 128, :].rearrange(
                                    "(rm c8) c -> c8 rm c", c8=8),
                                fo1[:, D * u:D * (u + 1)])
                    return run
                for u in range(4):
                    thunks.append(t_proj(u))

                def t_store():
                    if final:
                        return
                    xT, fo0, fo1 = make_epilogue.state
                    for b, fo in ((0, fo0), (1, fo1)):
                        eng = nc.sync if b == 0 else nc.gpsimd
                        for u in range(4):
                            r0 = CHUNK * n + 128 * u
                            eng.dma_start(
                                po[b, r0:r0 + 128, :].rearrange(
                                    "(rm c8) c -> c8 rm c", c8=8),
                                fo[:, D * u:D * (u + 1)])
                thunks.append(t_store)
                return thunks

            pending = []
            for n in (0, 1, 2, 3):
                n_t = 4 * n + 4
                psx0 = psX.tile([DK + 1, CHUNK], f32, tag="psx0")
                psx1 = psX.tile([DK + 1, CHUNK], f32, tag="psx1")
                for t_ in range(n_t):
                    s_ = t_ - 4 * n
                    off = max(0, s_) * JT
                    # b1-shift: b1's surviving cols start right at CHUNK,
                    # so exp covers one gap-free range [off : 2*CHUNK-off]
                    pss = psS.tile([128, 2 * CHUNK], f32, tag="pss")
                    nc.tensor.matmul(
                        pss[:, off:CHUNK],
                        qhT_stat(t_)[0:64, :],
                        qhT_mov(n, off, CHUNK)[0:64, :],
                        start=True, stop=True, tile_position=(0, 0))
                    nc.tensor.matmul(
                        pss[:, CHUNK:2 * CHUNK - off],
                        qhT_stat(t_)[64:128, :],
                        qhT_mov(n, off, CHUNK)[64:128, :],
                        start=True, stop=True, tile_position=(64, 0))
                    eT = eTp.tile([128, 2 * CHUNK], bf16, tag="eT")
                    nc.scalar.activation(
                        eT[:, off:2 * CHUNK - off], pss[:, off:2 * CHUNK - off],
                        mybir.ActivationFunctionType.Exp,
                        bias=exp_bias[:], scale=1.0 / math.sqrt(DK))
                    if s_ >= 0:
                        sl0 = eT[:, off:off + JT]
                        nc.gpsimd.tensor_mul(sl0, sl0, mt_sb[:])
                        sl1 = eT[:, CHUNK:CHUNK + JT]
                        nc.gpsimd.tensor_mul(sl1, sl1, mt_sb[:])
                    if n == 0 and t_ == 0:
                        # first-half v projection: after chunk 0's first
                        # score matmul (so exp starts ASAP) but ahead of
                        # the first PV in the PE's program order
                        vproj((0, 2))
                        vgather(0)
                    elif n == 1 and t_ == 0:
                        vproj((3,))
                        vgather(1)
                    elif n == 0 and t_ == 1:
                        qproj_half(1)
                    nc.tensor.matmul(
                        psx0[:, off:], vh_all[0][:, 65 * t_:65 * t_ + 65],
                        eT[:, off:CHUNK],
                        start=(t_ == 0), stop=(t_ == n_t - 1),
                        skip_group_check=True)
                    nc.tensor.matmul(
                        psx1[:, off:], vh_all[1][:, 65 * t_:65 * t_ + 65],
                        eT[:, CHUNK:2 * CHUNK - off],
                        start=(t_ == 0), stop=(t_ == n_t - 1),
                        skip_group_check=True)
                    if pending:
                        pending.pop(0)()
                if n == 0:
                    # second-half v projection split: rh1 here, rh3 inside
                    # chunk 1's first tile (smaller PE bursts)
                    vproj((1,))
                for th in pending:
                    th()
                pending = make_epilogue(n, psx0, psx1, final=(n == 3))
            for th in pending:
                th()
    nc.compile()
    return nc


def _mask_patterns():
    # stored index p' = c8*16 + rm holds true j/i offset 8*rm + c8
    import ml_dtypes
    x = np.arange(JT)
    pi = 8 * (x % 16) + x // 16
    return (pi[:, None] <= pi[None, :]).astype(ml_dtypes.bfloat16)


def _numpy_fallback(query, key, value, mask, Wq, Wk, Wv, Wo):
    q = query @ Wq.T
    v = value @ Wv.T
    qh = q.reshape(B, H, S, DK)
    vh = v.reshape(B, H, S, DK)
    scores = np.einsum("bhqd,bhkd->bhqk", qh, qh) / math.sqrt(DK)
    scores = np.where(mask == 0, np.float32(-1e9), scores)
    scores = scores - scores.max(axis=-1, keepdims=True)
    e = np.exp(scores)
    attn = e / e.sum(axis=-1, keepdims=True)
    x = np.einsum("bhqk,bhkd->bhqd", attn, vh)
    x = x.transpose(0, 2, 1, 3).reshape(B, S, H * DK)
    return (x @ Wo.T).astype(np.float32)


def _run_device(query, value, Wq, Wv, Wo, trace=False):
    import ml_dtypes
    from concourse.bass_utils import run_bass_kernel_spmd

    if "nc" not in _cache:
        _cache["nc"] = _build_causal()
    nc = _cache["nc"]

    bf = ml_dtypes.bfloat16
    mtri = _mask_patterns()
    wqT = np.ascontiguousarray(Wq.T).astype(bf)
    wvT = np.ascontiguousarray(Wv.T).astype(bf)
    in_maps = []
    for c in range(NCORES):
        r0 = SLAB * c
        qs = query[:, r0:r0 + SLAB, :]      # [B, SLAB, D]
        vs = value[:, r0:r0 + SLAB, :]
        in_maps.append({
            # [D, 2*SLAB]: b0 cols then b1 cols
            "qT": np.ascontiguousarray(
                qs.transpose(2, 0, 1).reshape(D, 2 * SLAB)).astype(bf),
            "vT": np.ascontiguousarray(
                vs.transpose(2, 0, 1).reshape(D, 2 * SLAB)).astype(bf),
            "wqT": wqT,
            "wvT": wvT,
            "woT": np.ascontiguousarray(
                Wo[:, DK * c:DK * (c + 1)].T).astype(bf),
            "mtri": mtri,
        })
    res = run_bass_kernel_spmd(nc, in_maps, core_ids=list(range(NCORES)),
                               trace=trace)
    out = np.zeros((B, S, D), dtype=np.float32)
    # lo comes back in stored order: s_stored = blk*128 + c8*16 + rm
    # holds true position blk*128 + 8*rm + c8
    x = np.arange(S)
    blk, off = x // 128, x % 128
    lperm = blk * 128 + (off % 8) * 16 + off // 8  # true_s -> stored_s
    for c in range(NCORES):
        pc = res.results[c]
        lo_true = pc["lo"].reshape(B, S)[:, lperm]
        out += pc["po"].astype(np.float32) / lo_true.reshape(B, S, 1)
    return out, res


_TRIL = None


def kernel(query, key, value, mask, Wq, Wk, Wv, Wo):
    global _TRIL
    query = np.asarray(query, dtype=np.float32)
    value = np.asarray(value, dtype=np.float32)
    mask = np.asarray(mask)
    Wq = np.asarray(Wq, dtype=np.float32)
    Wv = np.asarray(Wv, dtype=np.float32)
    Wo = np.asarray(Wo, dtype=np.float32)

    if _TRIL is None:
        _TRIL = np.tril(np.ones((S, S), dtype=np.int64))
    m2 = mask.reshape(S, S)
    if not np.array_equal(m2 != 0, _TRIL != 0):
        return _numpy_fallback(query, np.asarray(key), value, mask,
                               Wq, np.asarray(Wk), Wv, Wo)

    out, _ = _run_device(query, value, Wq, Wv, Wo)
    return out
